# revision 2
# baseline (speedup 1.0000x reference)
"""Trainium2 Bass kernel for causal multi-head attention (GPT-style block).

Reference computation (per batch b):
    qkv = x @ w_attn + b_attn ; q,k,v = split(qkv)
    per head: S = q k^T / sqrt(64); causal mask; P = softmax(S); a = P v
    out = concat_heads(a) @ w_proj + b_proj

Shapes: x (2, 2048, 1024), 16 heads, head_dim 64.

Sharding: 8 cores = 2 batches x 4 head-groups (4 heads each).  Tensor
parallel over heads; host sums the 4 head-group partial c_proj outputs
(bf16) per batch and adds b_proj.

Key optimizations vs the plain bf16 pipeline:
  * Q/K projections run in fp8(e4m3) DoubleRow: x*32 and w*1024 quantized
    on host; one DR matmul contracts 256 rows (measured ~264ns for 512 out
    cols, same as a 128-deep bf16 matmul), halving Q/K projection PE time.
    The 2^30 product scale folds into the softmax exp scale (0.125 * 2^-30,
    exact).  V projection stays bf16 for accuracy (fp8 V alone measured
    3e-2 end-to-end; QK-only measures 1.36e-2 vs the 2e-2 budget).
  * V is computed in natural [token, channel] layout (x^T tiles stationary,
    w_v moving), eliminating v1's PE transposes + F32R staging.
  * S blocks are [128, 512] with per-block exp on ScalarE (~630ns) to keep
    the S->exp->PV critical chain short.  Band-block masking: one
    affine_select per block on GpSimd covering dead prefix + triangle.
  * A dummy partition_broadcast at build time pulls the one-time ~6.5us
    GpSimd ucode library swap into the startup DMA shadow; without it the
    first chunk's softmax-denominator broadcasts queue behind the swap and
    stall the PE ~7us (and drop the PE clock to mid p-state).
  * Filler scheduling: next chunk's Q/K projections drain in the front 2/3
    of each chunk (its S blocks need QT/KT at the boundary), V mid-chunk,
    c_proj of older chunks late (chunk 3 runs cproj(1)+cproj(2) to stay
    PE-bound over the exp pacing); whole groups only, so at most two
    accumulation groups share the 2 filler PSUM banks.
  * c_proj partials are written in bf16 (half the output DMA).
  * x ships chunk-major (contiguous per-chunk DMA), fp8 and bf16 copies.
  * Filler pacing reserves a few ops to cover the end-of-chunk denominator
    window so the PE clock does not drop before the dense tail.
"""

import sys

sys.path.insert(0, "/opt/trn_rl_repo")

import numpy as np
import ml_dtypes

import concourse.bacc as bacc
import concourse.mybir as mybir
import concourse.tile as tile
from concourse.bass_utils import run_bass_kernel_spmd

F32 = mybir.dt.float32
BF16 = mybir.dt.bfloat16
FP8 = mybir.dt.float8e4
DR = mybir.MatmulPerfMode.DoubleRow
NP_BF16 = np.dtype(ml_dtypes.bfloat16)
NP_FP8 = np.dtype(ml_dtypes.float8_e4m3)

B = 2
T = 2048
NX = 1024
H = 16
HD = 64
NCORES = 8
NHG = 4          # head groups (cores per batch)
NH = 4           # heads per core
CW = NH * HD     # 256 channel width per core
QC = 512         # q-chunk
NQC = T // QC    # 4
KT = 128         # k-tile
VW = HD + 1      # V columns + ones column

SX = 32.0        # fp8 x scale (2^5)
SW = 1024.0      # fp8 w scale (2^10)
EXP_SCALE = 0.125 / (SX * SW) ** 2   # 2^-33, exact


def _build():
    nc = bacc.Bacc("TRN2", target_bir_lowering=False, debug=False,
                   num_devices=NCORES)
    x8_d = nc.dram_tensor("x8", [128, 8 * T], FP8, kind="ExternalInput")
    xb_d = nc.dram_tensor("xb", [128, 8 * T], BF16, kind="ExternalInput")
    w8_d = nc.dram_tensor("w8", [128, 4 * 4 * 2 * 128], FP8,
                          kind="ExternalInput")
    wv_d = nc.dram_tensor("wv", [128, 8 * CW], BF16, kind="ExternalInput")
    wp_d = nc.dram_tensor("wp", [128, 2 * NX], BF16, kind="ExternalInput")
    qkbias_d = nc.dram_tensor("qkbias", [128, 4], F32, kind="ExternalInput")
    vbias_d = nc.dram_tensor("vbias", [128, 2 * CW], BF16,
                             kind="ExternalInput")
    out_d = nc.dram_tensor("out_p", [T, NX], BF16, kind="ExternalOutput")

    Exp = mybir.ActivationFunctionType.Exp

    with tile.TileContext(nc) as tc:
        with (
            tc.tile_pool(name="pers", bufs=1) as pers,
            tc.tile_pool(name="ps", bufs=1, space="PSUM") as psum,
            tc.tile_pool(name="ptp", bufs=10) as ptp,
            tc.tile_pool(name="op", bufs=6) as op,
            tc.tile_pool(name="rp", bufs=6) as rp,
        ):
            # ---- persistent tiles; DMA order front-loads what the first
            # matmuls need.  Weights on the sync queue, x chunks on gpsimd
            # (chunk-major, contiguous per chunk).
            w8 = pers.tile([128, 4, 4, 2, 128], FP8, tag="w8")
            w8_ap = w8_d.ap().rearrange("p (g a i m) -> p g a i m",
                                        g=4, a=4, i=2)
            x8 = pers.tile([128, NQC, 8, QC], FP8, tag="x8")
            xb = pers.tile([128, NQC, 8, QC], BF16, tag="xb")
            x8_ap = x8_d.ap().rearrange("p (c j t) -> p c j t", c=NQC, j=8)
            xb_ap = xb_d.ap().rearrange("p (c j t) -> p c j t", c=NQC, j=8)
            # smallest-first so the first QK-DR matmul can start ASAP;
            # chunk-0 x on gpsimd (short queue), weights + later chunks on
            # the sync HWDGE queue
            nc.gpsimd.dma_start(x8[:, 0, 0:2], x8_ap[:, 0, 0:2])
            nc.sync.dma_start(w8[:, 0], w8_ap[:, 0])
            qkbias = pers.tile([128, 4], F32, tag="qkbias")
            nc.sync.dma_start(qkbias[:], qkbias_d.ap())
            nc.gpsimd.dma_start(x8[:, 0, 2:8], x8_ap[:, 0, 2:8])
            for g in range(1, 4):
                nc.sync.dma_start(w8[:, g], w8_ap[:, g])
            wv = pers.tile([128, 8, CW], BF16, tag="wv")
            nc.sync.dma_start(wv[:],
                              wv_d.ap().rearrange("p (j d) -> p j d", j=8))
            vbias = pers.tile([128, 2 * CW], BF16, tag="vbias")
            nc.sync.dma_start(vbias[:], vbias_d.ap())
            nc.gpsimd.dma_start(xb[:, 0, 0:4], xb_ap[:, 0, 0:4])
            nc.gpsimd.dma_start(xb[:, 0, 4:8], xb_ap[:, 0, 4:8])
            # tiny dummy broadcast right after chunk-0's posts: forces the
            # GpSimd ucode lib that contains partition_broadcast to load
            # during the startup shadow (one-time multi-us swap) instead of
            # stalling the first chunk's denominator chain.  Chunks 1-3 ride
            # the sync queue so nothing else queues behind the lib load.
            warm0 = pers.tile([1, 8], F32, tag="warm0")
            nc.vector.memset(warm0[:], 1.0)
            warm1 = pers.tile([64, 8], F32, tag="warm1")
            nc.gpsimd.partition_broadcast(warm1[:], warm0[:])
            wp = pers.tile([128, 2, NX], BF16, tag="wp")
            nc.sync.dma_start(wp[:],
                              wp_d.ap().rearrange("p (c n) -> p c n", c=2))
            for cc in range(1, NQC):
                nc.sync.dma_start(x8[:, cc], x8_ap[:, cc])
                nc.sync.dma_start(xb[:, cc], xb_ap[:, cc])

            QT = [pers.tile([128, T], BF16, tag=f"qt{i}", name=f"qt{i}")
                  for i in range(2)]
            KTs = [pers.tile([128, T], BF16, tag=f"kt{i}", name=f"kt{i}")
                   for i in range(2)]
            anorm = [pers.tile([128, T], BF16, tag=f"an{i}", name=f"an{i}")
                     for i in range(2)]
            # V natural layout + ones column per head
            vaug = pers.tile([128, T // KT, NH, VW], BF16, tag="vaug")
            nc.vector.memset(vaug[:, :, :, HD:HD + 1], 1.0)

            # ---- PSUM banks: 4 S blocks, 2 PV accums, 2 filler banks.
            def bank(i, shape, dtype=F32):
                return psum.tile(shape, dtype, tag=f"bank{i}", bufs=1,
                                 name=f"bank{i}")

            fl_state = {"toggle": 0}
            flb = [psum.tile([128, QC], F32, tag=f"fl{i}", bufs=1,
                             name=f"fl{i}") for i in range(2)]

            def fl_bank():
                fl_state["toggle"] ^= 1
                fl_state["bank"] = flb[fl_state["toggle"]]
                return fl_state["bank"]

            def qkv_ops(qq):
                """Filler ops for chunk qq: (QK fp8-DR groups, natural-V).
                QK ops are drained early (next chunk's S needs QT/KT);
                V ops mid; cproj late."""
                qs = slice(qq * QC, (qq + 1) * QC)
                ops = []
                vops = []
                g_cur = []
                for g in range(4):          # q0, q1, k0, k1
                    gs = {}
                    for a in range(4):
                        def f(g=g, a=a, qq=qq, gs=gs):
                            if a == 0:
                                gs["b"] = fl_bank()
                            fl = gs["b"]
                            nc.tensor.matmul(
                                fl[:], w8[:, g, a, :, :],
                                x8[:, qq, 2 * a:2 * a + 2, :],
                                start=(a == 0), stop=(a == 3), perf_mode=DR)
                            if a == 3:
                                dst = (QT if g < 2 else KTs)[g % 2][:, qs]
                                nc.vector.tensor_scalar_add(
                                    dst, fl[:], qkbias[:, g:g + 1])
                        ops.append(f)
                        g_cur.append(f)
                for tp in range(2):          # t-subtile pairs
                    gs = {}
                    g_v = []
                    for half in range(2):
                        for j in range(8):
                            def f(tp=tp, half=half, j=j, qq=qq, gs=gs):
                                if half == 0 and j == 0:
                                    gs["b"] = fl_bank()
                                fl = gs["b"]
                                t0 = (2 * tp + half) * 128
                                nc.tensor.matmul(
                                    fl[:, half * CW:(half + 1) * CW],
                                    xb[:, qq, j, t0:t0 + 128], wv[:, j, :],
                                    start=(j == 0), stop=(j == 7))
                                if half == 1 and j == 7:
                                    tt = qq * 4 + 2 * tp
                                    dst = vaug[:, tt:tt + 2, :, 0:HD]
                                    src = fl[:].rearrange(
                                        "p (u h d) -> p u h d", u=2, h=NH)
                                    nc.vector.scalar_tensor_tensor(
                                        dst, src, 1.0,
                                        vbias[:].rearrange(
                                            "p (u h d) -> p u h d",
                                            u=2, h=NH),
                                        op0=mybir.AluOpType.mult,
                                        op1=mybir.AluOpType.add)
                            g_v.append(f)
                    vops.append(g_v)
                return ops, vops

            def cproj_ops(qq, act_ok=False):
                """Filler op groups: output projection rows for chunk qq."""
                ops = []
                for i in range(4):
                    tt = qq * 4 + i
                    for nxh in range(2):
                        gs = {}
                        g_cp = []
                        for c2 in range(2):
                            def f(tt=tt, nxh=nxh, c2=c2, i=i, gs=gs):
                                if c2 == 0:
                                    gs["b"] = fl_bank()
                                po = gs["b"]
                                nc.tensor.matmul(
                                    po[:],
                                    anorm[c2][:, tt * 128:(tt + 1) * 128],
                                    wp[:, c2, nxh * QC:(nxh + 1) * QC],
                                    start=(c2 == 0), stop=(c2 == 1))
                                if c2 == 1:
                                    ot = op.tile([128, QC], BF16, tag="ot")
                                    if act_ok and (i * 2 + nxh) % 2 == 0:
                                        nc.scalar.copy(ot[:], po[:])
                                    else:
                                        nc.vector.tensor_copy(ot[:], po[:])
                                    nc.sync.dma_start(
                                        out_d.ap()[tt * 128:(tt + 1) * 128,
                                                   nxh * QC:(nxh + 1) * QC],
                                        ot[:])
                            g_cp.append(f)
                        ops.append(g_cp)
                return ops

            def attention_head(h, qq, drain, steps_left):
                """Head h for q-chunk qq; S/exp/PV k-tile pipeline
                interleaved with filler drain."""
                c2, hh = divmod(h, 2)
                rows = slice(64 * hh, 64 * hh + 64)
                nk = 4 * qq + 4
                qs = slice(qq * QC, (qq + 1) * QC)
                pa = bank(4 + h % 2, [VW, QC])
                pts = {}
                LA = 2

                def s_block(kk):
                    ps_s = bank(kk % 4, [128, QC])
                    nc.tensor.matmul(ps_s[:],
                                     KTs[c2][rows, kk * KT:(kk + 1) * KT],
                                     QT[c2][rows, qs], start=True, stop=True)
                    pt = ptp.tile([128, QC], BF16, tag="pt")
                    j = kk - 4 * qq
                    if j >= 0:
                        # band block: exp live cols, then one affine_select
                        # zeroing dead prefix + above-diagonal triangle
                        nc.scalar.activation(pt[:, 128 * j:QC],
                                             ps_s[:, 128 * j:QC], Exp,
                                             scale=EXP_SCALE)
                        nc.gpsimd.affine_select(
                            pt[:, 0:128 * (j + 1)], pt[:, 0:128 * (j + 1)],
                            pattern=[[1, 128 * (j + 1)]],
                            compare_op=mybir.AluOpType.is_ge, fill=0.0,
                            base=-128 * j, channel_multiplier=-1)
                    else:
                        nc.scalar.activation(pt[:], ps_s[:], Exp,
                                             scale=EXP_SCALE)
                    pts[kk] = pt

                def pv_block(kk):
                    nc.tensor.matmul(pa[:], vaug[:, kk, h, :],
                                     pts.pop(kk)[:],
                                     start=(kk == 0), stop=(kk == nk - 1))

                for kk in range(min(LA, nk)):
                    s_block(kk)
                for kk in range(nk):
                    if kk + LA < nk:
                        s_block(kk + LA)
                    drain()
                    pv_block(kk)

                dn = rp.tile([1, QC], F32, tag="dn")
                nc.vector.tensor_copy(dn[:], pa[HD:HD + 1, :])
                recip = rp.tile([1, QC], F32, tag="recip")
                nc.vector.reciprocal_approx_fast(recip[:], dn[:])
                rbc = rp.tile([64, QC], F32, tag="rbc")
                nc.gpsimd.partition_broadcast(rbc[:], recip[:])
                nc.vector.tensor_mul(anorm[c2][rows, qs], pa[0:HD, :], rbc[:])

            # ---- main pipeline ----
            # Filler placement keeps every chunk's PE oversubscribed vs the
            # ScalarE exp pacing (~780ns/step vs ~500ns/step of bare S+PV):
            #   qq=0: QK(1)+V(1)   qq=1: QK(2)+V(2)   qq=2: QK(3)+V(3)+cproj(0)
            #   qq=3: cproj(1)+cproj(2)        tail: cproj(3)
            # QK of the next chunk drains in the front 2/3 (next chunk's S
            # needs QT/KT at the boundary); the rest spreads evenly with
            # ~HOLD ops kept to bridge the last head's denominator chain.
            HOLD_G = 4   # slow grp held back to bridge the chunk boundary
            qk0, v0 = qkv_ops(0)
            for f in qk0:
                f()
            for grp in v0:
                for f in grp:
                    f()
            for qq in range(NQC):
                fast, slow = [], []
                if qq + 1 < NQC:
                    qk1, v1 = qkv_ops(qq + 1)
                    fast += qk1
                    slow += v1
                if qq == 2:
                    slow += cproj_ops(0)
                if qq == 3:
                    slow += cproj_ops(1) + cproj_ops(2)
                nsteps = NH * (4 * qq + 4)
                front = [max(1, (2 * nsteps) // 3)]
                steps_left = [nsteps]
                nslow_cl = sum(len(g) for g in slow)
                stride = max(1, nsteps // max(1, nslow_cl - 2 * HOLD_G))
                fpops = [0]

                def drain(fast=fast, slow=slow, front=front,
                          steps_left=steps_left, stride=stride,
                          nsteps=nsteps, fpops=fpops):
                    if fast:
                        n = -(-len(fast) // max(1, front[0]))
                        for _ in range(n):
                            if fast:
                                fast.pop(0)()
                                fpops[0] += 1
                    step = nsteps - steps_left[0]
                    # whole slow group, only at fast-group boundaries so at
                    # most two filler groups (2 banks) are ever in flight
                    due = (step % stride == 0 and len(slow) > HOLD_G) or \
                        len(slow) * 2 > HOLD_G * 2 + steps_left[0]
                    if due and fpops[0] % 4 == 0 and slow:
                        for f in slow.pop(0):
                            f()
                    front[0] -= 1
                    steps_left[0] -= 1

                for h in range(NH):
                    attention_head(h, qq, drain, steps_left)
                for f in fast:
                    f()
                for grp in slow:
                    for f in grp:
                        f()
            for grp in cproj_ops(NQC - 1, act_ok=True):
                for f in grp:
                    f()

    nc.compile()
    return nc


_CACHE = {}


def _get_nc():
    if "nc" not in _CACHE:
        _CACHE["nc"] = _build()
    return _CACHE["nc"]


def kernel(x, w_attn, b_attn, w_proj, b_proj):
    x = np.asarray(x, dtype=np.float32)
    w_attn = np.asarray(w_attn, dtype=np.float32)
    b_attn = np.asarray(b_attn, dtype=np.float32)
    w_proj = np.asarray(w_proj, dtype=np.float32)
    b_proj = np.asarray(b_proj, dtype=np.float32)

    in_maps = []
    for core in range(NCORES):
        b, hg = divmod(core, NHG)
        cols = slice(hg * CW, (hg + 1) * CW)
        xT = np.ascontiguousarray(x[b].T)               # [1024, T]
        # chunk-major: [128, chunk, j, 512]
        xj = xT.reshape(8, 128, NQC, QC).transpose(1, 2, 0, 3)

        # w8: [128, g, a, i, m] for groups q0,q1,k0,k1
        wq = w_attn[:, cols]
        wk = w_attn[:, NX:2 * NX][:, cols]
        w8 = np.empty((128, 4, 4, 2, 128), dtype=NP_FP8)
        for g, wg in enumerate([wq[:, :128], wq[:, 128:],
                                wk[:, :128], wk[:, 128:]]):
            # contraction index = a*256 + i*128 + p
            w8[:, g] = (wg * SW).reshape(4, 2, 128, 128).transpose(
                2, 0, 1, 3).astype(NP_FP8)
        wvn = w_attn[:, 2 * NX:][:, cols]               # [1024, 256]

        qkbias = np.zeros((128, 4), dtype=np.float32)
        bq = b_attn[cols]
        bk = b_attn[NX:2 * NX][cols]
        qkbias[:, 0] = bq[:128] * (SX * SW)
        qkbias[:, 1] = bq[128:] * (SX * SW)
        qkbias[:, 2] = bk[:128] * (SX * SW)
        qkbias[:, 3] = bk[128:] * (SX * SW)
        bv = b_attn[2 * NX:][cols]                      # [256]
        vbias = np.broadcast_to(np.tile(bv, 2), (128, 2 * CW))

        in_maps.append({
            "x8": (xj * SX).astype(NP_FP8).reshape(128, -1).copy(),
            "xb": xj.astype(NP_BF16).reshape(128, -1).copy(),
            "w8": w8.reshape(128, -1).copy(),
            "wv": wvn.reshape(8, 128, CW).transpose(1, 0, 2)
                .astype(NP_BF16).reshape(128, -1).copy(),
            "wp": np.ascontiguousarray(w_proj[cols, :]).astype(NP_BF16)
                .reshape(2, 128, NX).transpose(1, 0, 2).reshape(128, -1)
                .copy(),
            "qkbias": qkbias,
            "vbias": np.ascontiguousarray(vbias).astype(NP_BF16),
        })

    nc = _get_nc()
    res = run_bass_kernel_spmd(nc, in_maps, core_ids=list(range(NCORES)))
    _CACHE["last_res"] = res
    out = np.empty((B, T, NX), dtype=np.float32)
    for b in range(B):
        acc = res.results[b * NHG]["out_p"].astype(np.float32)
        for hg in range(1, NHG):
            acc = acc + res.results[b * NHG + hg]["out_p"].astype(np.float32)
        out[b] = acc + b_proj
    return out


# revision 3
# speedup vs baseline: 1.0020x; 1.0020x over previous
"""Trainium2 Bass kernel for causal multi-head attention (GPT-style block).

Reference computation (per batch b):
    qkv = x @ w_attn + b_attn ; q,k,v = split(qkv)
    per head: S = q k^T / sqrt(64); causal mask; P = softmax(S); a = P v
    out = concat_heads(a) @ w_proj + b_proj

Shapes: x (2, 2048, 1024), 16 heads, head_dim 64.

Sharding: 8 cores = 2 batches x 4 head-groups (4 heads each).  Tensor
parallel over heads; host sums the 4 head-group partial c_proj outputs
(bf16) per batch and adds b_proj.

Key optimizations vs the plain bf16 pipeline:
  * Q/K projections run in fp8(e4m3) DoubleRow: x*32 and w*1024 quantized
    on host; one DR matmul contracts 256 rows (measured ~264ns for 512 out
    cols, same as a 128-deep bf16 matmul), halving Q/K projection PE time.
    The 2^30 product scale folds into the softmax exp scale (0.125 * 2^-30,
    exact).  V projection stays bf16 for accuracy (fp8 V alone measured
    3e-2 end-to-end; QK-only measures 1.36e-2 vs the 2e-2 budget).
  * V is computed in natural [token, channel] layout (x^T tiles stationary,
    w_v moving), eliminating v1's PE transposes + F32R staging.
  * S blocks are [128, 512] with per-block exp on ScalarE (~630ns) to keep
    the S->exp->PV critical chain short.  Band-block masking: one
    affine_select per block on GpSimd covering dead prefix + triangle.
  * A dummy partition_broadcast at build time pulls the one-time ~6.5us
    GpSimd ucode library swap into the startup DMA shadow; without it the
    first chunk's softmax-denominator broadcasts queue behind the swap and
    stall the PE ~7us (and drop the PE clock to mid p-state).
  * Filler scheduling keeps the PE oversubscribed vs ScalarE's exp pacing:
    next chunk's Q/K projections drain in the front 2/3 of each chunk (its
    S blocks need QT/KT at the boundary), V mid-chunk, c_proj of chunks
    0-2 spread through chunk 3; whole groups only, so at most two
    accumulation groups share the 2 filler PSUM banks; ~8 groups are held
    back and issued inside the last head BEFORE its denominator-chain
    writes (issued after, the dependency tracker orders their anorm reads
    behind the write and the PE idles through the chain).
  * c_proj partials are written in bf16 (half the output DMA).
  * x ships chunk-major (contiguous per-chunk DMA), fp8 and bf16 copies.
  * Filler pacing reserves a few ops to cover the end-of-chunk denominator
    window so the PE clock does not drop before the dense tail.
"""

import sys

sys.path.insert(0, "/opt/trn_rl_repo")

import numpy as np
import ml_dtypes

import concourse.bacc as bacc
import concourse.mybir as mybir
import concourse.tile as tile
from concourse.bass_utils import run_bass_kernel_spmd

F32 = mybir.dt.float32
BF16 = mybir.dt.bfloat16
FP8 = mybir.dt.float8e4
DR = mybir.MatmulPerfMode.DoubleRow
NP_BF16 = np.dtype(ml_dtypes.bfloat16)
NP_FP8 = np.dtype(ml_dtypes.float8_e4m3)

B = 2
T = 2048
NX = 1024
H = 16
HD = 64
NCORES = 8
NHG = 4          # head groups (cores per batch)
NH = 4           # heads per core
CW = NH * HD     # 256 channel width per core
QC = 512         # q-chunk
NQC = T // QC    # 4
KT = 128         # k-tile
VW = HD + 1      # V columns + ones column

SX = 32.0        # fp8 x scale (2^5)
SW = 1024.0      # fp8 w scale (2^10)
EXP_SCALE = 0.125 / (SX * SW) ** 2   # 2^-33, exact


def _build():
    nc = bacc.Bacc("TRN2", target_bir_lowering=False, debug=False,
                   num_devices=NCORES)
    x8_d = nc.dram_tensor("x8", [128, 8 * T], FP8, kind="ExternalInput")
    xb_d = nc.dram_tensor("xb", [128, 8 * T], BF16, kind="ExternalInput")
    w8_d = nc.dram_tensor("w8", [128, 4 * 4 * 2 * 128], FP8,
                          kind="ExternalInput")
    wv_d = nc.dram_tensor("wv", [128, 8 * CW], BF16, kind="ExternalInput")
    wp_d = nc.dram_tensor("wp", [128, 2 * NX], BF16, kind="ExternalInput")
    qkbias_d = nc.dram_tensor("qkbias", [128, 4], F32, kind="ExternalInput")
    vbias_d = nc.dram_tensor("vbias", [128, 2 * CW], BF16,
                             kind="ExternalInput")
    out_d = nc.dram_tensor("out_p", [T, NX], BF16, kind="ExternalOutput")

    Exp = mybir.ActivationFunctionType.Exp

    with tile.TileContext(nc) as tc:
        with (
            tc.tile_pool(name="pers", bufs=1) as pers,
            tc.tile_pool(name="ps", bufs=1, space="PSUM") as psum,
            tc.tile_pool(name="ptp", bufs=10) as ptp,
            tc.tile_pool(name="op", bufs=6) as op,
            tc.tile_pool(name="rp", bufs=6) as rp,
        ):
            # ---- persistent tiles; DMA order front-loads what the first
            # matmuls need.  Weights on the sync queue, x chunks on gpsimd
            # (chunk-major, contiguous per chunk).
            w8 = pers.tile([128, 4, 4, 2, 128], FP8, tag="w8")
            w8_ap = w8_d.ap().rearrange("p (g a i m) -> p g a i m",
                                        g=4, a=4, i=2)
            x8 = pers.tile([128, NQC, 8, QC], FP8, tag="x8")
            xb = pers.tile([128, NQC, 8, QC], BF16, tag="xb")
            x8_ap = x8_d.ap().rearrange("p (c j t) -> p c j t", c=NQC, j=8)
            xb_ap = xb_d.ap().rearrange("p (c j t) -> p c j t", c=NQC, j=8)
            # smallest-first so the first QK-DR matmul can start ASAP;
            # chunk-0 x on gpsimd (short queue), weights + later chunks on
            # the sync HWDGE queue
            nc.gpsimd.dma_start(x8[:, 0, 0:2], x8_ap[:, 0, 0:2])
            nc.sync.dma_start(w8[:, 0], w8_ap[:, 0])
            qkbias = pers.tile([128, 4], F32, tag="qkbias")
            nc.sync.dma_start(qkbias[:], qkbias_d.ap())
            nc.gpsimd.dma_start(x8[:, 0, 2:8], x8_ap[:, 0, 2:8])
            for g in range(1, 4):
                nc.sync.dma_start(w8[:, g], w8_ap[:, g])
            wv = pers.tile([128, 8, CW], BF16, tag="wv")
            nc.sync.dma_start(wv[:],
                              wv_d.ap().rearrange("p (j d) -> p j d", j=8))
            vbias = pers.tile([128, 2 * CW], BF16, tag="vbias")
            nc.sync.dma_start(vbias[:], vbias_d.ap())
            nc.gpsimd.dma_start(xb[:, 0, 0:4], xb_ap[:, 0, 0:4])
            nc.gpsimd.dma_start(xb[:, 0, 4:8], xb_ap[:, 0, 4:8])
            # tiny dummy broadcast right after chunk-0's posts: forces the
            # GpSimd ucode lib that contains partition_broadcast to load
            # during the startup shadow (one-time multi-us swap) instead of
            # stalling the first chunk's denominator chain.  Chunks 1-3 ride
            # the sync queue so nothing else queues behind the lib load.
            warm0 = pers.tile([1, 8], F32, tag="warm0")
            nc.vector.memset(warm0[:], 1.0)
            warm1 = pers.tile([64, 8], F32, tag="warm1")
            nc.gpsimd.partition_broadcast(warm1[:], warm0[:])
            wp = pers.tile([128, 2, NX], BF16, tag="wp")
            nc.sync.dma_start(wp[:],
                              wp_d.ap().rearrange("p (c n) -> p c n", c=2))
            for cc in range(1, NQC):
                nc.sync.dma_start(x8[:, cc], x8_ap[:, cc])
                nc.sync.dma_start(xb[:, cc], xb_ap[:, cc])

            QT = [pers.tile([128, T], BF16, tag=f"qt{i}", name=f"qt{i}")
                  for i in range(2)]
            KTs = [pers.tile([128, T], BF16, tag=f"kt{i}", name=f"kt{i}")
                   for i in range(2)]
            anorm = [pers.tile([128, T], BF16, tag=f"an{i}", name=f"an{i}")
                     for i in range(2)]
            # V natural layout + ones column per head
            vaug = pers.tile([128, T // KT, NH, VW], BF16, tag="vaug")
            nc.vector.memset(vaug[:, :, :, HD:HD + 1], 1.0)

            # ---- PSUM banks: 4 S blocks, 2 PV accums, 2 filler banks.
            def bank(i, shape, dtype=F32):
                return psum.tile(shape, dtype, tag=f"bank{i}", bufs=1,
                                 name=f"bank{i}")

            fl_state = {"toggle": 0}
            flb = [psum.tile([128, QC], F32, tag=f"fl{i}", bufs=1,
                             name=f"fl{i}") for i in range(2)]

            def fl_bank():
                fl_state["toggle"] ^= 1
                fl_state["bank"] = flb[fl_state["toggle"]]
                return fl_state["bank"]

            def qkv_ops(qq):
                """Filler ops for chunk qq: (QK fp8-DR groups, natural-V).
                QK ops are drained early (next chunk's S needs QT/KT);
                V ops mid; cproj late."""
                qs = slice(qq * QC, (qq + 1) * QC)
                ops = []
                vops = []
                g_cur = []
                for g in range(4):          # q0, q1, k0, k1
                    gs = {}
                    for a in range(4):
                        def f(g=g, a=a, qq=qq, gs=gs):
                            if a == 0:
                                gs["b"] = fl_bank()
                            fl = gs["b"]
                            nc.tensor.matmul(
                                fl[:], w8[:, g, a, :, :],
                                x8[:, qq, 2 * a:2 * a + 2, :],
                                start=(a == 0), stop=(a == 3), perf_mode=DR)
                            if a == 3:
                                dst = (QT if g < 2 else KTs)[g % 2][:, qs]
                                nc.vector.tensor_scalar_add(
                                    dst, fl[:], qkbias[:, g:g + 1])
                        ops.append(f)
                        g_cur.append(f)
                for tp in range(2):          # t-subtile pairs
                    gs = {}
                    g_v = []
                    for half in range(2):
                        for j in range(8):
                            def f(tp=tp, half=half, j=j, qq=qq, gs=gs):
                                if half == 0 and j == 0:
                                    gs["b"] = fl_bank()
                                fl = gs["b"]
                                t0 = (2 * tp + half) * 128
                                nc.tensor.matmul(
                                    fl[:, half * CW:(half + 1) * CW],
                                    xb[:, qq, j, t0:t0 + 128], wv[:, j, :],
                                    start=(j == 0), stop=(j == 7))
                                if half == 1 and j == 7:
                                    tt = qq * 4 + 2 * tp
                                    dst = vaug[:, tt:tt + 2, :, 0:HD]
                                    src = fl[:].rearrange(
                                        "p (u h d) -> p u h d", u=2, h=NH)
                                    nc.vector.scalar_tensor_tensor(
                                        dst, src, 1.0,
                                        vbias[:].rearrange(
                                            "p (u h d) -> p u h d",
                                            u=2, h=NH),
                                        op0=mybir.AluOpType.mult,
                                        op1=mybir.AluOpType.add)
                            g_v.append(f)
                    vops.append(g_v)
                return ops, vops

            def cproj_ops(qq, act_ok=False):
                """Filler op groups: output projection rows for chunk qq."""
                ops = []
                for i in range(4):
                    tt = qq * 4 + i
                    for nxh in range(2):
                        gs = {}
                        g_cp = []
                        for c2 in range(2):
                            def f(tt=tt, nxh=nxh, c2=c2, i=i, gs=gs):
                                if c2 == 0:
                                    gs["b"] = fl_bank()
                                po = gs["b"]
                                nc.tensor.matmul(
                                    po[:],
                                    anorm[c2][:, tt * 128:(tt + 1) * 128],
                                    wp[:, c2, nxh * QC:(nxh + 1) * QC],
                                    start=(c2 == 0), stop=(c2 == 1))
                                if c2 == 1:
                                    ot = op.tile([128, QC], BF16, tag="ot")
                                    if act_ok and (i * 2 + nxh) % 2 == 0:
                                        nc.scalar.copy(ot[:], po[:])
                                    else:
                                        nc.vector.tensor_copy(ot[:], po[:])
                                    nc.sync.dma_start(
                                        out_d.ap()[tt * 128:(tt + 1) * 128,
                                                   nxh * QC:(nxh + 1) * QC],
                                        ot[:])
                            g_cp.append(f)
                        ops.append(g_cp)
                return ops

            def attention_head(h, qq, drain, steps_left,
                                   tail_drain=None):
                """Head h for q-chunk qq; S/exp/PV k-tile pipeline
                interleaved with filler drain."""
                c2, hh = divmod(h, 2)
                rows = slice(64 * hh, 64 * hh + 64)
                nk = 4 * qq + 4
                qs = slice(qq * QC, (qq + 1) * QC)
                pa = bank(4 + h % 2, [VW, QC])
                pts = {}
                LA = 2

                def s_block(kk):
                    ps_s = bank(kk % 4, [128, QC])
                    nc.tensor.matmul(ps_s[:],
                                     KTs[c2][rows, kk * KT:(kk + 1) * KT],
                                     QT[c2][rows, qs], start=True, stop=True)
                    pt = ptp.tile([128, QC], BF16, tag="pt")
                    j = kk - 4 * qq
                    if j >= 0:
                        # band block: exp live cols, then one affine_select
                        # zeroing dead prefix + above-diagonal triangle
                        nc.scalar.activation(pt[:, 128 * j:QC],
                                             ps_s[:, 128 * j:QC], Exp,
                                             scale=EXP_SCALE)
                        nc.gpsimd.affine_select(
                            pt[:, 0:128 * (j + 1)], pt[:, 0:128 * (j + 1)],
                            pattern=[[1, 128 * (j + 1)]],
                            compare_op=mybir.AluOpType.is_ge, fill=0.0,
                            base=-128 * j, channel_multiplier=-1)
                    else:
                        nc.scalar.activation(pt[:], ps_s[:], Exp,
                                             scale=EXP_SCALE)
                    pts[kk] = pt

                def pv_block(kk):
                    nc.tensor.matmul(pa[:], vaug[:, kk, h, :],
                                     pts.pop(kk)[:],
                                     start=(kk == 0), stop=(kk == nk - 1))

                for kk in range(min(LA, nk)):
                    s_block(kk)
                for kk in range(nk):
                    if kk + LA < nk:
                        s_block(kk + LA)
                    drain()
                    pv_block(kk)

                if tail_drain is not None:
                    # issue leftover fillers BEFORE the denominator-chain
                    # writes: issued after them, the dependency tracker
                    # orders their anorm reads behind this head's anorm
                    # write and the PE idles through the whole chain
                    tail_drain()
                dn = rp.tile([1, QC], F32, tag="dn")
                nc.vector.tensor_copy(dn[:], pa[HD:HD + 1, :])
                recip = rp.tile([1, QC], F32, tag="recip")
                nc.vector.reciprocal_approx_fast(recip[:], dn[:])
                rbc = rp.tile([64, QC], F32, tag="rbc")
                nc.gpsimd.partition_broadcast(rbc[:], recip[:])
                nc.vector.tensor_mul(anorm[c2][rows, qs], pa[0:HD, :], rbc[:])

            # ---- main pipeline ----
            # Filler placement keeps every chunk's PE oversubscribed vs the
            # ScalarE exp pacing (~780ns/step vs ~500ns/step of bare S+PV):
            #   qq=0: QK(1)+V(1)   qq=1: QK(2)+V(2)   qq=2: QK(3)+V(3)+cproj(0)
            #   qq=3: cproj(1)+cproj(2)        tail: cproj(3)
            # QK of the next chunk drains in the front 2/3 (next chunk's S
            # needs QT/KT at the boundary); the rest spreads evenly with
            # ~HOLD ops kept to bridge the last head's denominator chain.
            HOLD_G = 8   # slow grps held back to bridge the chunk boundary
            qk0, v0 = qkv_ops(0)
            for f in qk0:
                f()
            for grp in v0:
                for f in grp:
                    f()
            for qq in range(NQC):
                fast, slow = [], []
                if qq + 1 < NQC:
                    qk1, v1 = qkv_ops(qq + 1)
                    fast += qk1
                    slow += v1
                if qq == 3:
                    slow += cproj_ops(0) + cproj_ops(1) + cproj_ops(2)
                nsteps = NH * (4 * qq + 4)
                front = [max(1, (2 * nsteps) // 3)]
                steps_left = [nsteps]
                stride = max(1, nsteps // max(1, len(slow) - HOLD_G))
                fpops = [0]

                def drain(fast=fast, slow=slow, front=front,
                          steps_left=steps_left, stride=stride,
                          nsteps=nsteps, fpops=fpops):
                    if fast:
                        n = -(-len(fast) // max(1, front[0]))
                        for _ in range(n):
                            if fast:
                                fast.pop(0)()
                                fpops[0] += 1
                    step = nsteps - steps_left[0]
                    # whole slow group, only at fast-group boundaries so at
                    # most two filler groups (2 banks) are ever in flight
                    due = (step % stride == 0 and len(slow) > HOLD_G) or \
                        len(slow) * 2 > HOLD_G * 2 + steps_left[0]
                    if due and fpops[0] % 4 == 0 and slow:
                        for f in slow.pop(0):
                            f()
                    front[0] -= 1
                    steps_left[0] -= 1

                def drain_all(fast=fast, slow=slow):
                    for f in fast:
                        f()
                    fast.clear()
                    for grp in slow:
                        for f in grp:
                            f()
                    slow.clear()

                for h in range(NH):
                    attention_head(h, qq, drain, steps_left,
                                   tail_drain=(drain_all if h == NH - 1
                                               else None))
            for grp in cproj_ops(NQC - 1, act_ok=True):
                for f in grp:
                    f()

    nc.compile()
    return nc


_CACHE = {}


def _get_nc():
    if "nc" not in _CACHE:
        _CACHE["nc"] = _build()
    return _CACHE["nc"]


def kernel(x, w_attn, b_attn, w_proj, b_proj):
    x = np.asarray(x, dtype=np.float32)
    w_attn = np.asarray(w_attn, dtype=np.float32)
    b_attn = np.asarray(b_attn, dtype=np.float32)
    w_proj = np.asarray(w_proj, dtype=np.float32)
    b_proj = np.asarray(b_proj, dtype=np.float32)

    in_maps = []
    for core in range(NCORES):
        b, hg = divmod(core, NHG)
        cols = slice(hg * CW, (hg + 1) * CW)
        xT = np.ascontiguousarray(x[b].T)               # [1024, T]
        # chunk-major: [128, chunk, j, 512]
        xj = xT.reshape(8, 128, NQC, QC).transpose(1, 2, 0, 3)

        # w8: [128, g, a, i, m] for groups q0,q1,k0,k1
        wq = w_attn[:, cols]
        wk = w_attn[:, NX:2 * NX][:, cols]
        w8 = np.empty((128, 4, 4, 2, 128), dtype=NP_FP8)
        for g, wg in enumerate([wq[:, :128], wq[:, 128:],
                                wk[:, :128], wk[:, 128:]]):
            # contraction index = a*256 + i*128 + p
            w8[:, g] = (wg * SW).reshape(4, 2, 128, 128).transpose(
                2, 0, 1, 3).astype(NP_FP8)
        wvn = w_attn[:, 2 * NX:][:, cols]               # [1024, 256]

        qkbias = np.zeros((128, 4), dtype=np.float32)
        bq = b_attn[cols]
        bk = b_attn[NX:2 * NX][cols]
        qkbias[:, 0] = bq[:128] * (SX * SW)
        qkbias[:, 1] = bq[128:] * (SX * SW)
        qkbias[:, 2] = bk[:128] * (SX * SW)
        qkbias[:, 3] = bk[128:] * (SX * SW)
        bv = b_attn[2 * NX:][cols]                      # [256]
        vbias = np.broadcast_to(np.tile(bv, 2), (128, 2 * CW))

        in_maps.append({
            "x8": (xj * SX).astype(NP_FP8).reshape(128, -1).copy(),
            "xb": xj.astype(NP_BF16).reshape(128, -1).copy(),
            "w8": w8.reshape(128, -1).copy(),
            "wv": wvn.reshape(8, 128, CW).transpose(1, 0, 2)
                .astype(NP_BF16).reshape(128, -1).copy(),
            "wp": np.ascontiguousarray(w_proj[cols, :]).astype(NP_BF16)
                .reshape(2, 128, NX).transpose(1, 0, 2).reshape(128, -1)
                .copy(),
            "qkbias": qkbias,
            "vbias": np.ascontiguousarray(vbias).astype(NP_BF16),
        })

    nc = _get_nc()
    res = run_bass_kernel_spmd(nc, in_maps, core_ids=list(range(NCORES)))
    _CACHE["last_res"] = res
    out = np.empty((B, T, NX), dtype=np.float32)
    for b in range(B):
        acc = res.results[b * NHG]["out_p"].astype(np.float32)
        for hg in range(1, NHG):
            acc = acc + res.results[b * NHG + hg]["out_p"].astype(np.float32)
        out[b] = acc + b_proj
    return out


# revision 4
# speedup vs baseline: 1.0023x; 1.0003x over previous
"""Trainium2 Bass kernel for causal multi-head attention (GPT-style block).

Reference computation (per batch b):
    qkv = x @ w_attn + b_attn ; q,k,v = split(qkv)
    per head: S = q k^T / sqrt(64); causal mask; P = softmax(S); a = P v
    out = concat_heads(a) @ w_proj + b_proj

Shapes: x (2, 2048, 1024), 16 heads, head_dim 64.

Sharding: 8 cores = 2 batches x 4 head-groups (4 heads each).  Tensor
parallel over heads; host sums the 4 head-group partial c_proj outputs
(bf16) per batch and adds b_proj.

Key optimizations vs the plain bf16 pipeline:
  * Q/K projections run in fp8(e4m3) DoubleRow: x*32 and w*1024 quantized
    on host; one DR matmul contracts 256 rows (measured ~264ns for 512 out
    cols, same as a 128-deep bf16 matmul), halving Q/K projection PE time.
    The 2^30 product scale folds into the softmax exp scale (0.125 * 2^-30,
    exact).  V projection stays bf16 for accuracy (fp8 V alone measured
    3e-2 end-to-end; QK-only measures 1.36e-2 vs the 2e-2 budget).
  * V is computed in natural [token, channel] layout (x^T tiles stationary,
    w_v moving), eliminating v1's PE transposes + F32R staging.
  * S blocks are [128, 512] with per-block exp on ScalarE (~630ns) to keep
    the S->exp->PV critical chain short.  Band-block masking: one
    affine_select per block on GpSimd covering dead prefix + triangle.
  * A dummy partition_broadcast at build time pulls the one-time ~6.5us
    GpSimd ucode library swap into the startup DMA shadow; without it the
    first chunk's softmax-denominator broadcasts queue behind the swap and
    stall the PE ~7us (and drop the PE clock to mid p-state).
  * Chunk-0 x DMAs post from the ScalarE queue (preamble done ~4.7us and
    idle until the first exp), weights + later chunks ride the sync HWDGE
    queue, GpSimd carries only the ucode warmup - data lands earlier on
    all three fronts.
  * Filler scheduling keeps the PE oversubscribed vs ScalarE's exp pacing:
    next chunk's Q/K projections drain in the front 2/3 of each chunk (its
    S blocks need QT/KT at the boundary), V mid-chunk, c_proj of chunks
    0-2 spread through chunk 3; whole groups only, so at most two
    accumulation groups share the 2 filler PSUM banks; ~8 groups are held
    back and issued inside the last head BEFORE its denominator-chain
    writes (issued after, the dependency tracker orders their anorm reads
    behind the write and the PE idles through the chain).
  * c_proj partials are written in bf16 (half the output DMA).
  * x ships chunk-major (contiguous per-chunk DMA), fp8 and bf16 copies.
  * Filler pacing reserves a few ops to cover the end-of-chunk denominator
    window so the PE clock does not drop before the dense tail.
"""

import sys

sys.path.insert(0, "/opt/trn_rl_repo")

import numpy as np
import ml_dtypes

import concourse.bacc as bacc
import concourse.mybir as mybir
import concourse.tile as tile
from concourse.bass_utils import run_bass_kernel_spmd

F32 = mybir.dt.float32
BF16 = mybir.dt.bfloat16
FP8 = mybir.dt.float8e4
DR = mybir.MatmulPerfMode.DoubleRow
NP_BF16 = np.dtype(ml_dtypes.bfloat16)
NP_FP8 = np.dtype(ml_dtypes.float8_e4m3)

B = 2
T = 2048
NX = 1024
H = 16
HD = 64
NCORES = 8
NHG = 4          # head groups (cores per batch)
NH = 4           # heads per core
CW = NH * HD     # 256 channel width per core
QC = 512         # q-chunk
NQC = T // QC    # 4
KT = 128         # k-tile
VW = HD + 1      # V columns + ones column

SX = 32.0        # fp8 x scale (2^5)
SW = 1024.0      # fp8 w scale (2^10)
EXP_SCALE = 0.125 / (SX * SW) ** 2   # 2^-33, exact


def _build():
    nc = bacc.Bacc("TRN2", target_bir_lowering=False, debug=False,
                   num_devices=NCORES)
    x8_d = nc.dram_tensor("x8", [128, 8 * T], FP8, kind="ExternalInput")
    xb_d = nc.dram_tensor("xb", [128, 8 * T], BF16, kind="ExternalInput")
    w8_d = nc.dram_tensor("w8", [128, 4 * 4 * 2 * 128], FP8,
                          kind="ExternalInput")
    wv_d = nc.dram_tensor("wv", [128, 8 * CW], BF16, kind="ExternalInput")
    wp_d = nc.dram_tensor("wp", [128, 2 * NX], BF16, kind="ExternalInput")
    qkbias_d = nc.dram_tensor("qkbias", [128, 4], F32, kind="ExternalInput")
    vbias_d = nc.dram_tensor("vbias", [128, 2 * CW], BF16,
                             kind="ExternalInput")
    out_d = nc.dram_tensor("out_p", [T, NX], BF16, kind="ExternalOutput")

    Exp = mybir.ActivationFunctionType.Exp

    with tile.TileContext(nc) as tc:
        with (
            tc.tile_pool(name="pers", bufs=1) as pers,
            tc.tile_pool(name="ps", bufs=1, space="PSUM") as psum,
            tc.tile_pool(name="ptp", bufs=10) as ptp,
            tc.tile_pool(name="op", bufs=6) as op,
            tc.tile_pool(name="rp", bufs=6) as rp,
        ):
            # ---- persistent tiles; DMA order front-loads what the first
            # matmuls need.  Weights on the sync queue, x chunks on gpsimd
            # (chunk-major, contiguous per chunk).
            w8 = pers.tile([128, 4, 4, 2, 128], FP8, tag="w8")
            w8_ap = w8_d.ap().rearrange("p (g a i m) -> p g a i m",
                                        g=4, a=4, i=2)
            x8 = pers.tile([128, NQC, 8, QC], FP8, tag="x8")
            xb = pers.tile([128, NQC, 8, QC], BF16, tag="xb")
            x8_ap = x8_d.ap().rearrange("p (c j t) -> p c j t", c=NQC, j=8)
            xb_ap = xb_d.ap().rearrange("p (c j t) -> p c j t", c=NQC, j=8)
            # smallest-first so the first QK-DR matmul can start ASAP;
            # chunk-0 x on gpsimd (short queue), weights + later chunks on
            # the sync HWDGE queue
            nc.scalar.dma_start(x8[:, 0, 0:2], x8_ap[:, 0, 0:2])
            nc.sync.dma_start(w8[:, 0], w8_ap[:, 0])
            qkbias = pers.tile([128, 4], F32, tag="qkbias")
            nc.sync.dma_start(qkbias[:], qkbias_d.ap())
            nc.scalar.dma_start(x8[:, 0, 2:8], x8_ap[:, 0, 2:8])
            for g in range(1, 4):
                nc.sync.dma_start(w8[:, g], w8_ap[:, g])
            wv = pers.tile([128, 8, CW], BF16, tag="wv")
            nc.sync.dma_start(wv[:],
                              wv_d.ap().rearrange("p (j d) -> p j d", j=8))
            vbias = pers.tile([128, 2 * CW], BF16, tag="vbias")
            nc.sync.dma_start(vbias[:], vbias_d.ap())
            nc.scalar.dma_start(xb[:, 0, 0:4], xb_ap[:, 0, 0:4])
            nc.scalar.dma_start(xb[:, 0, 4:8], xb_ap[:, 0, 4:8])
            # tiny dummy broadcast right after chunk-0's posts: forces the
            # GpSimd ucode lib that contains partition_broadcast to load
            # during the startup shadow (one-time multi-us swap) instead of
            # stalling the first chunk's denominator chain.  Chunks 1-3 ride
            # the sync queue so nothing else queues behind the lib load.
            warm0 = pers.tile([1, 8], F32, tag="warm0")
            nc.vector.memset(warm0[:], 1.0)
            warm1 = pers.tile([64, 8], F32, tag="warm1")
            nc.gpsimd.partition_broadcast(warm1[:], warm0[:])
            wp = pers.tile([128, 2, NX], BF16, tag="wp")
            nc.sync.dma_start(wp[:],
                              wp_d.ap().rearrange("p (c n) -> p c n", c=2))
            for cc in range(1, NQC):
                nc.sync.dma_start(x8[:, cc], x8_ap[:, cc])
                nc.sync.dma_start(xb[:, cc], xb_ap[:, cc])

            QT = [pers.tile([128, T], BF16, tag=f"qt{i}", name=f"qt{i}")
                  for i in range(2)]
            KTs = [pers.tile([128, T], BF16, tag=f"kt{i}", name=f"kt{i}")
                   for i in range(2)]
            anorm = [pers.tile([128, T], BF16, tag=f"an{i}", name=f"an{i}")
                     for i in range(2)]
            # V natural layout + ones column per head
            vaug = pers.tile([128, T // KT, NH, VW], BF16, tag="vaug")
            nc.vector.memset(vaug[:, :, :, HD:HD + 1], 1.0)

            # ---- PSUM banks: 4 S blocks, 2 PV accums, 2 filler banks.
            def bank(i, shape, dtype=F32):
                return psum.tile(shape, dtype, tag=f"bank{i}", bufs=1,
                                 name=f"bank{i}")

            fl_state = {"toggle": 0}
            flb = [psum.tile([128, QC], F32, tag=f"fl{i}", bufs=1,
                             name=f"fl{i}") for i in range(2)]

            def fl_bank():
                fl_state["toggle"] ^= 1
                fl_state["bank"] = flb[fl_state["toggle"]]
                return fl_state["bank"]

            def qkv_ops(qq):
                """Filler ops for chunk qq: (QK fp8-DR groups, natural-V).
                QK ops are drained early (next chunk's S needs QT/KT);
                V ops mid; cproj late."""
                qs = slice(qq * QC, (qq + 1) * QC)
                ops = []
                vops = []
                g_cur = []
                for g in range(4):          # q0, q1, k0, k1
                    gs = {}
                    for a in range(4):
                        def f(g=g, a=a, qq=qq, gs=gs):
                            if a == 0:
                                gs["b"] = fl_bank()
                            fl = gs["b"]
                            nc.tensor.matmul(
                                fl[:], w8[:, g, a, :, :],
                                x8[:, qq, 2 * a:2 * a + 2, :],
                                start=(a == 0), stop=(a == 3), perf_mode=DR)
                            if a == 3:
                                dst = (QT if g < 2 else KTs)[g % 2][:, qs]
                                nc.vector.tensor_scalar_add(
                                    dst, fl[:], qkbias[:, g:g + 1])
                        ops.append(f)
                        g_cur.append(f)
                for tp in range(2):          # t-subtile pairs
                    gs = {}
                    g_v = []
                    for half in range(2):
                        for j in range(8):
                            def f(tp=tp, half=half, j=j, qq=qq, gs=gs):
                                if half == 0 and j == 0:
                                    gs["b"] = fl_bank()
                                fl = gs["b"]
                                t0 = (2 * tp + half) * 128
                                nc.tensor.matmul(
                                    fl[:, half * CW:(half + 1) * CW],
                                    xb[:, qq, j, t0:t0 + 128], wv[:, j, :],
                                    start=(j == 0), stop=(j == 7))
                                if half == 1 and j == 7:
                                    tt = qq * 4 + 2 * tp
                                    dst = vaug[:, tt:tt + 2, :, 0:HD]
                                    src = fl[:].rearrange(
                                        "p (u h d) -> p u h d", u=2, h=NH)
                                    nc.vector.scalar_tensor_tensor(
                                        dst, src, 1.0,
                                        vbias[:].rearrange(
                                            "p (u h d) -> p u h d",
                                            u=2, h=NH),
                                        op0=mybir.AluOpType.mult,
                                        op1=mybir.AluOpType.add)
                            g_v.append(f)
                    vops.append(g_v)
                return ops, vops

            def cproj_ops(qq, act_ok=False):
                """Filler op groups: output projection rows for chunk qq."""
                ops = []
                for i in range(4):
                    tt = qq * 4 + i
                    for nxh in range(2):
                        gs = {}
                        g_cp = []
                        for c2 in range(2):
                            def f(tt=tt, nxh=nxh, c2=c2, i=i, gs=gs):
                                if c2 == 0:
                                    gs["b"] = fl_bank()
                                po = gs["b"]
                                nc.tensor.matmul(
                                    po[:],
                                    anorm[c2][:, tt * 128:(tt + 1) * 128],
                                    wp[:, c2, nxh * QC:(nxh + 1) * QC],
                                    start=(c2 == 0), stop=(c2 == 1))
                                if c2 == 1:
                                    ot = op.tile([128, QC], BF16, tag="ot")
                                    if act_ok and (i * 2 + nxh) % 2 == 0:
                                        nc.scalar.copy(ot[:], po[:])
                                    else:
                                        nc.vector.tensor_copy(ot[:], po[:])
                                    nc.sync.dma_start(
                                        out_d.ap()[tt * 128:(tt + 1) * 128,
                                                   nxh * QC:(nxh + 1) * QC],
                                        ot[:])
                            g_cp.append(f)
                        ops.append(g_cp)
                return ops

            def attention_head(h, qq, drain, steps_left,
                                   tail_drain=None):
                """Head h for q-chunk qq; S/exp/PV k-tile pipeline
                interleaved with filler drain."""
                c2, hh = divmod(h, 2)
                rows = slice(64 * hh, 64 * hh + 64)
                nk = 4 * qq + 4
                qs = slice(qq * QC, (qq + 1) * QC)
                pa = bank(4 + h % 2, [VW, QC])
                pts = {}
                LA = 2

                def s_block(kk):
                    ps_s = bank(kk % 4, [128, QC])
                    nc.tensor.matmul(ps_s[:],
                                     KTs[c2][rows, kk * KT:(kk + 1) * KT],
                                     QT[c2][rows, qs], start=True, stop=True)
                    pt = ptp.tile([128, QC], BF16, tag="pt")
                    j = kk - 4 * qq
                    if j >= 0:
                        # band block: exp live cols, then one affine_select
                        # zeroing dead prefix + above-diagonal triangle
                        nc.scalar.activation(pt[:, 128 * j:QC],
                                             ps_s[:, 128 * j:QC], Exp,
                                             scale=EXP_SCALE)
                        nc.gpsimd.affine_select(
                            pt[:, 0:128 * (j + 1)], pt[:, 0:128 * (j + 1)],
                            pattern=[[1, 128 * (j + 1)]],
                            compare_op=mybir.AluOpType.is_ge, fill=0.0,
                            base=-128 * j, channel_multiplier=-1)
                    else:
                        nc.scalar.activation(pt[:], ps_s[:], Exp,
                                             scale=EXP_SCALE)
                    pts[kk] = pt

                def pv_block(kk):
                    nc.tensor.matmul(pa[:], vaug[:, kk, h, :],
                                     pts.pop(kk)[:],
                                     start=(kk == 0), stop=(kk == nk - 1))

                for kk in range(min(LA, nk)):
                    s_block(kk)
                for kk in range(nk):
                    if kk + LA < nk:
                        s_block(kk + LA)
                    drain()
                    pv_block(kk)

                if tail_drain is not None:
                    # issue leftover fillers BEFORE the denominator-chain
                    # writes: issued after them, the dependency tracker
                    # orders their anorm reads behind this head's anorm
                    # write and the PE idles through the whole chain
                    tail_drain()
                dn = rp.tile([1, QC], F32, tag="dn")
                nc.vector.tensor_copy(dn[:], pa[HD:HD + 1, :])
                recip = rp.tile([1, QC], F32, tag="recip")
                nc.vector.reciprocal_approx_fast(recip[:], dn[:])
                rbc = rp.tile([64, QC], F32, tag="rbc")
                nc.gpsimd.partition_broadcast(rbc[:], recip[:])
                nc.vector.tensor_mul(anorm[c2][rows, qs], pa[0:HD, :], rbc[:])

            # ---- main pipeline ----
            # Filler placement keeps every chunk's PE oversubscribed vs the
            # ScalarE exp pacing (~780ns/step vs ~500ns/step of bare S+PV):
            #   qq=0: QK(1)+V(1)   qq=1: QK(2)+V(2)   qq=2: QK(3)+V(3)+cproj(0)
            #   qq=3: cproj(1)+cproj(2)        tail: cproj(3)
            # QK of the next chunk drains in the front 2/3 (next chunk's S
            # needs QT/KT at the boundary); the rest spreads evenly with
            # ~HOLD ops kept to bridge the last head's denominator chain.
            HOLD_G = 8   # slow grps held back to bridge the chunk boundary
            qk0, v0 = qkv_ops(0)
            for f in qk0:
                f()
            for grp in v0:
                for f in grp:
                    f()
            for qq in range(NQC):
                fast, slow = [], []
                if qq + 1 < NQC:
                    qk1, v1 = qkv_ops(qq + 1)
                    fast += qk1
                    slow += v1
                if qq == 3:
                    slow += cproj_ops(0) + cproj_ops(1) + cproj_ops(2)
                nsteps = NH * (4 * qq + 4)
                front = [max(1, (2 * nsteps) // 3)]
                steps_left = [nsteps]
                stride = max(1, nsteps // max(1, len(slow) - HOLD_G))
                fpops = [0]

                def drain(fast=fast, slow=slow, front=front,
                          steps_left=steps_left, stride=stride,
                          nsteps=nsteps, fpops=fpops):
                    if fast:
                        n = -(-len(fast) // max(1, front[0]))
                        for _ in range(n):
                            if fast:
                                fast.pop(0)()
                                fpops[0] += 1
                    step = nsteps - steps_left[0]
                    # whole slow group, only at fast-group boundaries so at
                    # most two filler groups (2 banks) are ever in flight
                    due = (step % stride == 0 and len(slow) > HOLD_G) or \
                        len(slow) * 2 > HOLD_G * 2 + steps_left[0]
                    if due and fpops[0] % 4 == 0 and slow:
                        for f in slow.pop(0):
                            f()
                    front[0] -= 1
                    steps_left[0] -= 1

                def drain_all(fast=fast, slow=slow):
                    for f in fast:
                        f()
                    fast.clear()
                    for grp in slow:
                        for f in grp:
                            f()
                    slow.clear()

                for h in range(NH):
                    attention_head(h, qq, drain, steps_left,
                                   tail_drain=(drain_all if h == NH - 1
                                               else None))
            for grp in cproj_ops(NQC - 1, act_ok=True):
                for f in grp:
                    f()

    nc.compile()
    return nc


_CACHE = {}


def _get_nc():
    if "nc" not in _CACHE:
        _CACHE["nc"] = _build()
    return _CACHE["nc"]


def kernel(x, w_attn, b_attn, w_proj, b_proj):
    x = np.asarray(x, dtype=np.float32)
    w_attn = np.asarray(w_attn, dtype=np.float32)
    b_attn = np.asarray(b_attn, dtype=np.float32)
    w_proj = np.asarray(w_proj, dtype=np.float32)
    b_proj = np.asarray(b_proj, dtype=np.float32)

    in_maps = []
    for core in range(NCORES):
        b, hg = divmod(core, NHG)
        cols = slice(hg * CW, (hg + 1) * CW)
        xT = np.ascontiguousarray(x[b].T)               # [1024, T]
        # chunk-major: [128, chunk, j, 512]
        xj = xT.reshape(8, 128, NQC, QC).transpose(1, 2, 0, 3)

        # w8: [128, g, a, i, m] for groups q0,q1,k0,k1
        wq = w_attn[:, cols]
        wk = w_attn[:, NX:2 * NX][:, cols]
        w8 = np.empty((128, 4, 4, 2, 128), dtype=NP_FP8)
        for g, wg in enumerate([wq[:, :128], wq[:, 128:],
                                wk[:, :128], wk[:, 128:]]):
            # contraction index = a*256 + i*128 + p
            w8[:, g] = (wg * SW).reshape(4, 2, 128, 128).transpose(
                2, 0, 1, 3).astype(NP_FP8)
        wvn = w_attn[:, 2 * NX:][:, cols]               # [1024, 256]

        qkbias = np.zeros((128, 4), dtype=np.float32)
        bq = b_attn[cols]
        bk = b_attn[NX:2 * NX][cols]
        qkbias[:, 0] = bq[:128] * (SX * SW)
        qkbias[:, 1] = bq[128:] * (SX * SW)
        qkbias[:, 2] = bk[:128] * (SX * SW)
        qkbias[:, 3] = bk[128:] * (SX * SW)
        bv = b_attn[2 * NX:][cols]                      # [256]
        vbias = np.broadcast_to(np.tile(bv, 2), (128, 2 * CW))

        in_maps.append({
            "x8": (xj * SX).astype(NP_FP8).reshape(128, -1).copy(),
            "xb": xj.astype(NP_BF16).reshape(128, -1).copy(),
            "w8": w8.reshape(128, -1).copy(),
            "wv": wvn.reshape(8, 128, CW).transpose(1, 0, 2)
                .astype(NP_BF16).reshape(128, -1).copy(),
            "wp": np.ascontiguousarray(w_proj[cols, :]).astype(NP_BF16)
                .reshape(2, 128, NX).transpose(1, 0, 2).reshape(128, -1)
                .copy(),
            "qkbias": qkbias,
            "vbias": np.ascontiguousarray(vbias).astype(NP_BF16),
        })

    nc = _get_nc()
    res = run_bass_kernel_spmd(nc, in_maps, core_ids=list(range(NCORES)))
    _CACHE["last_res"] = res
    out = np.empty((B, T, NX), dtype=np.float32)
    for b in range(B):
        acc = res.results[b * NHG]["out_p"].astype(np.float32)
        for hg in range(1, NHG):
            acc = acc + res.results[b * NHG + hg]["out_p"].astype(np.float32)
        out[b] = acc + b_proj
    return out
